# revision 31
# baseline (speedup 1.0000x reference)
"""Bahdanau-attention MOMA (3-branch additive attention) Trainium2 kernel.

Problem (hardcoded shapes): S=1024, B=32, QD=VD=E=1024.
  per branch i:  k = value @ Wv_i.T            [S,B,E]   (dominant compute)
                 scores = sum_e vn_i[e] * tanh(pq_i[b,e] + k[s,b,e])
                 a_i = softmax_s(scores)
  A = sum_i gamma_i * a_i                      [S,B]  (outputs 2,3)
  context = einsum('sb,sbd->bd', A, value)     [B,VD]  (output 1)

Distribution: batch B=32 sharded over 8 NeuronCores (4 batch rows per core,
fully independent -> no collectives). Host precomputes the cheap small-tensor
math (pq = query@Wq.T + b, normed v) and pre-transposes/casts value to bf16 in
the layouts the tensor engine needs; the device does the three big matmuls,
tanh, the v-reduction, softmax, and the context reduction.

Engine PSUM/SBUF accesses must start at 32-aligned partitions, so per-batch-row
reductions (the e-contraction "matvec" and the context reduction) run as four
concurrent 32-wide PE column strips via tile_position, with batch row b kept on
partition 32*b ("strip layout"). Softmax is per-partition and therefore works
directly on strips; the PE transpose then moves strip rows into free-dim
columns where strided extraction is unrestricted. Matvec groups are software-
pipelined two e-tiles behind the main matmuls so the four strip matmuls issue
back-to-back (concurrent on hardware).
"""

import os
import sys

import numpy as np

for _p in ("/opt/trn_rl_repo",):
    if os.path.isdir(_p) and _p not in sys.path:
        sys.path.insert(0, _p)

import ml_dtypes

import concourse.bass as bass
import concourse.mybir as mybir
import concourse.tile as tile
from concourse import bacc
from concourse.bass_utils import run_bass_kernel_spmd
from concourse.masks import make_identity

S, B, QD, VD, E = 1024, 32, 1024, 1024, 1024
N_CORES = 8
BL = B // N_CORES          # batch rows per core = 4
P = 128                    # partitions
DT = VD // P               # 8 d-tiles
ET = E // P                # 8 e-tiles
ST = S // P                # 8 s-tiles
BF16 = mybir.dt.bfloat16
F32 = mybir.dt.float32
AF = mybir.ActivationFunctionType


def build_program(gammas, use_mask):
    """Emit the per-core SPMD Bass/Tile program. gammas are baked immediates."""
    nc = bacc.Bacc("TRN2", target_bir_lowering=False, debug=False)

    # vt free layout: f = sh*(BL*512) + b*512 + (s % 512)  (s-half major) so a
    # pass's four chunks are contiguous and can be DMA'd ahead of the rest.
    vt_d = nc.dram_tensor("vt", [DT, P, BL * S], BF16, kind="ExternalInput")
    vnat_d = nc.dram_tensor("vnat", [BL, ST, P, VD], BF16, kind="ExternalInput")
    wvt_d = nc.dram_tensor("wvt", [3, DT, P, E], BF16, kind="ExternalInput")
    pqb_d = nc.dram_tensor("pqb", [P, 3, BL, ET], F32, kind="ExternalInput")
    vnr_d = nc.dram_tensor("vnr", [P, 3, ET, BL], BF16, kind="ExternalInput")
    if use_mask:
        # strip layout: row 32*b carries mask bias for batch row b
        mb_d = nc.dram_tensor("mb", [P, S], BF16, kind="ExternalInput")
    # host-computed (G/S) * sum_s value[s, b, :] in fp32 — the exact "uniform
    # attention" part of the context; the device only adds the small
    # correction sum_s (A - G/S) * value, so bf16 noise is scaled down ~15x.
    vsum_d = nc.dram_tensor("vsum", [BL, VD], F32, kind="ExternalInput")
    ctx_d = nc.dram_tensor("ctx", [BL, VD], F32, kind="ExternalOutput")
    attn_d = nc.dram_tensor("attn", [ST, P, BL], F32, kind="ExternalOutput")

    with tile.TileContext(nc) as tc:
        with (
            tc.tile_pool(name="big", bufs=1) as big,
            tc.tile_pool(name="tpool", bufs=12) as tpool,
            tc.tile_pool(name="small", bufs=1) as small,
            tc.tile_pool(name="stats", bufs=4) as stats,
            tc.tile_pool(name="pk", bufs=6, space="PSUM") as pk_pool,
            tc.tile_pool(name="ps", bufs=2, space="PSUM") as ps_pool,
        ):
            # ---- resident loads ----
            vt = big.tile([P, DT, BL * S], BF16)
            wvt = big.tile([P, 3, DT, E], BF16)
            pqb = big.tile([P, 3, BL, ET], F32)
            vnr = big.tile([P, 3, ET, BL], BF16)
            # DMA priority order = first-consumption order: branch-0 weights
            # for the first e-tiles (one transfer), then the vt pass-0 halves
            # per d-tile (pipelined arrival), then everything else in larger
            # consolidated transfers (per-dma issue overhead is ~0.6us).
            nc.sync.dma_start(
                wvt[:, 0, :, : E // 2],
                wvt_d[0].rearrange("dt p e -> p dt e")[:, :, : E // 2],
            )
            half = BL * 512
            for dt in range(DT):
                nc.sync.dma_start(vt[:, dt, :half], vt_d[dt, :, :half])
            nc.sync.dma_start(pqb[:], pqb_d[:])
            nc.sync.dma_start(vnr[:], vnr_d[:])
            nc.sync.dma_start(
                wvt[:, 0, :, E // 2 :],
                wvt_d[0].rearrange("dt p e -> p dt e")[:, :, E // 2 :],
            )
            for dt in range(DT):
                nc.sync.dma_start(vt[:, dt, half:], vt_d[dt, :, half:])
            for i in (1, 2):
                nc.sync.dma_start(
                    wvt[:, i, :, :], wvt_d[i].rearrange("dt p e -> p dt e")
                )
            if use_mask:
                mb = big.tile([P, S], BF16)
                nc.sync.dma_start(mb[:], mb_d[:])
            vnat = big.tile([P, BL, ST, VD], BF16)

            # Strip layout: batch row b lives on partition 32*b; rows in
            # between hold matvec replicas / garbage and are never read back
            # (the PE transpose moves them to free-dim columns we skip).
            A = big.tile([P, S], F32)           # gamma-weighted attention
            ident = big.tile([P, P], F32)
            make_identity(nc, ident[:])

            # ---- per-branch: k-matmul + tanh + v-reduction + softmax ----
            for i in range(3):
                scores = small.tile([P, S], F32, tag="scores")
                for pss in range(2):          # pass p = s-half p, chunks = 4 b's
                    # per-pass scores psum: strip 32*b <- batch row b
                    psum_s = ps_pool.tile([P, 512], F32, tag="ps_s",
                                          name=f"ps_s_{i}_{pss}")
                    # matvec groups are delayed 2 e-tiles so the 4 strip MMs
                    # issue adjacently (-> concurrent column strips) with
                    # their tanh inputs already complete.
                    pending = []

                    def flush(pend):
                        for et_, cc_, t_ in pend:
                            nc.tensor.matmul(
                                psum_s[32 * cc_ : 32 * cc_ + BL, :],
                                vnr[:, i, et_, :],
                                t_,
                                start=(et_ == 0),
                                stop=(et_ == ET - 1),
                                tile_position=(0, 32 * cc_),
                            )

                    for et in range(ET):
                        if len(pending) >= 8:
                            flush(pending[:4])
                            pending = pending[4:]
                        psum_k = [
                            pk_pool.tile([P, 512], F32, tag="ps_k",
                                         name=f"ps_k_{i}_{pss}_{et}_{j}")
                            for j in range(4)
                        ]
                        for dt in range(DT):
                            lhs = wvt[:, i, dt, et * P : (et + 1) * P]
                            for cc in range(4):
                                nc.tensor.matmul(
                                    psum_k[cc][:],
                                    lhs,
                                    vt[:, dt,
                                       pss * 4 * 512 + cc * 512
                                       : pss * 4 * 512 + (cc + 1) * 512],
                                    start=(dt == 0),
                                    stop=(dt == DT - 1),
                                )
                        # tanh(pq[b, e-tile] + k) -> bf16 t tile
                        for cc in range(4):
                            t_t = tpool.tile([P, 512], BF16, tag="t",
                                             name=f"t_{i}_{pss}_{et}_{cc}")
                            nc.scalar.activation(
                                t_t[:],
                                psum_k[cc][:],
                                AF.Tanh,
                                bias=pqb[:, i, cc, et : et + 1],
                            )
                            pending.append((et, cc, t_t[:]))
                    flush(pending)
                    pending = []
                    nc.vector.tensor_copy(
                        scores[:, pss * 512 : (pss + 1) * 512], psum_s[:]
                    )

                # prefetch the phase-B value copy during branch-0 compute
                if i == 0:
                    for bl in range(BL):
                        for st in range(ST):
                            nc.sync.dma_start(vnat[:, bl, st, :], vnat_d[bl, st])

                # masked softmax over s (free dim); all ops are row-local so
                # the garbage rows between strips are harmless.
                if use_mask:
                    nc.vector.tensor_add(scores[:], scores[:], mb[:])
                nmx = stats.tile([P, 1], F32, tag="nmx", name=f"nmx{i}")
                nc.vector.reduce_max(
                    out=nmx[:], in_=scores[:], axis=mybir.AxisListType.X, negate=True
                )
                zsum = stats.tile([P, 1], F32, tag="z", name=f"z{i}")
                nc.scalar.activation(
                    scores[:], scores[:], AF.Exp, bias=nmx[:], accum_out=zsum[:]
                )
                rz = stats.tile([P, 1], F32, tag="rz", name=f"rz{i}")
                nc.vector.reciprocal(rz[:], zsum[:])
                nc.vector.tensor_scalar_mul(rz[:], rz[:], float(gammas[i]))
                if i == 0:
                    nc.vector.tensor_scalar_mul(A[:], scores[:], rz[:])
                else:
                    nc.vector.tensor_scalar_mul(scores[:], scores[:], rz[:])
                    nc.vector.tensor_add(A[:], A[:], scores[:])

            # ---- A^T (via PE transpose) -> attn output + phase-B stationary ----
            gsum = float(sum(gammas))
            at_f = big.tile([P, ST, BL], F32)
            da_f = big.tile([P, ST, BL], F32)     # A^T - G/S
            # da_rep[:, st, bl, :]: dA^T[:, bl] replicated in all 4 columns
            da_rep = big.tile([P, ST, BL, BL], BF16)
            for st in range(ST):
                pt = ps_pool.tile([P, P], F32, tag="ps_s", name=f"pt{st}")
                nc.tensor.transpose(pt[:], A[:, st * P : (st + 1) * P], ident[:])
                # keep only columns {0,32,64,96} = the real batch rows
                nc.vector.tensor_copy(
                    at_f[:, st, :],
                    pt[:].rearrange("p (c r) -> p c r", r=32)[:, :, 0],
                )
            nc.vector.tensor_scalar_add(da_f[:], at_f[:], -gsum / S)
            for bl in range(BL):
                nc.vector.tensor_copy(
                    da_rep[:, :, bl, :],
                    da_f[:, :, bl : bl + 1].to_broadcast((P, ST, BL)),
                )
            for st in range(ST):
                nc.sync.dma_start(attn_d[st], at_f[:, st, :])

            # ---- phase B: context = (G/S)*sum_s v + sum_s dA[s,b]*value[s,b,:] ----
            # strip layout again: context row b on partition 32*b; 4 batch
            # rows run as concurrent column-strip matmuls.
            ctx_sb = small.tile([P, VD], F32, tag="ctx")
            ctx_rows = ctx_sb[:].rearrange("(c r) d -> c r d", r=32)[:, 0, :]
            nc.sync.dma_start(ctx_rows, vsum_d[:])   # preload the mean term
            for hc in range(2):
                pc = ps_pool.tile([P, 512], F32, tag="ps_s", name=f"pc{hc}")
                for st in range(ST):
                    for bl in range(BL):
                        nc.tensor.matmul(
                            pc[32 * bl : 32 * bl + BL, :],
                            da_rep[:, st, bl, :],
                            vnat[:, bl, st, hc * 512 : (hc + 1) * 512],
                            start=(st == 0),
                            stop=(st == ST - 1),
                            tile_position=(0, 32 * bl),
                        )
                nc.vector.tensor_add(
                    ctx_sb[:, hc * 512 : (hc + 1) * 512],
                    pc[:],
                    ctx_sb[:, hc * 512 : (hc + 1) * 512],
                )
            nc.sync.dma_start(ctx_d[:], ctx_rows)

    nc.compile()
    return nc


def prep_inputs(query, value, key_padding_mask, Wqs, Wvs, vs, bs, gs, gammas):
    """Host-side prep: small math + per-core slicing/packing/casting."""
    bf = ml_dtypes.bfloat16
    gsum = float(sum(gammas))
    vsum_all = (gsum / S) * value.sum(axis=0, dtype=np.float64)  # [B, VD]
    vsum_all = vsum_all.astype(np.float32)
    # pq[i] = query @ Wq_i.T + b_i   [B, E]
    pq = np.stack(
        [query @ Wqs[i].T + bs[i][None, :] for i in range(3)], axis=0
    ).astype(np.float32)  # [3, B, E]
    vn = np.stack(
        [gs[i] * vs[i] / np.linalg.norm(vs[i]) for i in range(3)], axis=0
    ).astype(np.float32)  # [3, E]

    wvt = np.ascontiguousarray(
        np.stack([Wvs[i].T for i in range(3)], axis=0)
    ).astype(bf).reshape(3, DT, P, E)
    # vnr[p, i, et, j] = vn_i[et*128+p]  (replicated over the 4 M-columns)
    vn_pet = vn.reshape(3, ET, P).transpose(2, 0, 1)  # [P, 3, ET]
    vnr = np.ascontiguousarray(
        np.broadcast_to(vn_pet[:, :, :, None], (P, 3, ET, BL))
    ).astype(bf)

    use_mask = bool(np.asarray(key_padding_mask).any())
    in_maps = []
    for c in range(N_CORES):
        b0 = c * BL
        vsl = value[:, b0 : b0 + BL, :]                                 # [S,BL,VD]
        # vt free layout: [VD, sh(2), b(4), 512] (s-half major)
        vt = np.ascontiguousarray(
            vsl.transpose(2, 1, 0).reshape(VD, BL, 2, 512).transpose(0, 2, 1, 3)
        ).astype(bf)
        vnat = np.ascontiguousarray(vsl.transpose(1, 0, 2)).astype(bf)  # [BL,S,VD]
        # pqb[p, i, bl, et] = pq[i, b0+bl, et*128+p]
        pqb = np.ascontiguousarray(
            pq[:, b0 : b0 + BL, :].reshape(3, BL, ET, P).transpose(3, 0, 1, 2)
        ).astype(np.float32)
        m = {
            "vt": vt.reshape(VD, BL * S).reshape(DT, P, BL * S),
            "vnat": vnat.reshape(BL, ST, P, VD),
            "wvt": wvt,
            "pqb": pqb,
            "vnr": vnr,
            "vsum": np.ascontiguousarray(vsum_all[b0 : b0 + BL]),
        }
        if use_mask:
            msl = np.asarray(key_padding_mask)[:, b0 : b0 + BL]   # [S, BL]
            mbc = np.zeros((P, S), np.float32)
            mbc[::32, :] = np.where(msl.T, -1e30, 0.0)
            m["mb"] = mbc.astype(ml_dtypes.bfloat16)
        in_maps.append(m)
    return in_maps, use_mask


def kernel(query, value, key_padding_mask,
           Wq1, Wv1, v1, b1, g1, gamma1,
           Wq2, Wv2, v2, b2, g2, gamma2,
           Wq3, Wv3, v3, b3, g3, gamma3,
           _bench=None):
    query = np.asarray(query, np.float32)
    value = np.asarray(value, np.float32)
    gammas = [float(np.asarray(g).reshape(-1)[0]) for g in (gamma1, gamma2, gamma3)]
    in_maps, use_mask = prep_inputs(
        query, value, key_padding_mask,
        [np.asarray(w, np.float32) for w in (Wq1, Wq2, Wq3)],
        [np.asarray(w, np.float32) for w in (Wv1, Wv2, Wv3)],
        [np.asarray(v, np.float32) for v in (v1, v2, v3)],
        [np.asarray(b, np.float32) for b in (b1, b2, b3)],
        [float(np.asarray(g).reshape(-1)[0]) for g in (g1, g2, g3)],
        gammas,
    )
    nc = build_program(gammas, use_mask)
    if _bench is not None:
        _bench["nc"] = nc
        _bench["in_maps"] = in_maps
    res = run_bass_kernel_spmd(nc, in_maps, core_ids=list(range(N_CORES)))

    context = np.concatenate(
        [res.results[c]["ctx"] for c in range(N_CORES)], axis=0
    ).astype(np.float32)  # [B, VD]
    attn = np.concatenate(
        [res.results[c]["attn"].reshape(S, BL) for c in range(N_CORES)], axis=1
    ).astype(np.float32)  # [S, B]
    return context, attn, attn
